# revision 11
# baseline (speedup 1.0000x reference)
# Binary (sign) matmul: out[b,m,n] = sum_k sign(x[b,m,k]) * sign(y[b,n,k]) * x_clip * y_clip
# B=2, M=N=K=4096, fp32 in/out.
#
# Sharding: 8 cores = batch(2) x 2x2 grid over (M, N). Each core computes a
# [2048, 2048] output block from x[b, mh*2048:, :] and y[b, nh*2048:, :].
#
# Per-core device pipeline:
#   DMA fp32 row tiles -> ScalarE Sign -> bf16 +-1 -> TensorE transpose
#   (128x128 blocks via identity) -> PSUM -> DVE copy/cast into K-major fp8
#   operand buffers -> TensorE DoubleRow fp8 matmul (exact: sums of +-1
#   accumulate in fp32 PSUM) -> ScalarE scale by x_clip*y_clip (computed on
#   device) -> DMA out.
#
# Schedule: only the first N-quarter of sign(y)^T is built up front; the
# matmul phase runs as 4 passes over output column chunks with the x
# operands persisted in SBUF, so passes 1-3 overlap the remaining y prep.
import numpy as np

B = 2
M = N = K = 4096
P = 128
MSH, NSH = 2048, 2048      # per-core shard of M, N
KO = K // P                # 32 k-tiles of 128
NHALF = 2                  # staging splits each 4096-wide row in halves
KH = K // NHALF            # 2048
KOH = KO // NHALF          # 16 k-tiles per half
KQ = 8                     # transpose blocks packed per PSUM tile (1 bank)
MT = MSH // P              # 16 m row-tiles
NT = NSH // P              # 16 n row-tiles
FD = 512                   # matmul free dim
NCH = NSH // FD            # 4 n chunks (passes)
JPC = NT // NCH            # 4 y row-tiles per n chunk
NCORES = 8

USE_FP8 = True             # fp8e4 operands + DoubleRow perf mode


def _build_program():
    import concourse.bacc as bacc
    import concourse.mybir as mybir
    import concourse.tile as tile
    from concourse.bass import ts
    from concourse.masks import make_identity

    f32 = mybir.dt.float32
    bf16 = mybir.dt.bfloat16
    op_dt = mybir.dt.float8e4 if USE_FP8 else bf16
    Sign = mybir.ActivationFunctionType.Sign
    Copy = mybir.ActivationFunctionType.Copy

    # Bacc (not bass.Bass): its compile() legalizes multi-sem waits into
    # event-semaphore carriers — TRN2 instructions support only 1 HW wait.
    nc = bacc.Bacc(
        "TRN2",
        target_bir_lowering=False,
        debug=False,
        num_devices=NCORES,
    )
    xs = nc.dram_tensor("xs", [MSH, K], f32, kind="ExternalInput").ap()
    ys = nc.dram_tensor("ys", [NSH, K], f32, kind="ExternalInput").ap()
    clips = nc.dram_tensor("clips", [P, 2], f32, kind="ExternalInput").ap()
    out = nc.dram_tensor("out", [MSH, NSH], f32, kind="ExternalOutput").ap()

    with tile.TileContext(nc) as tc:
        with (
            tc.tile_pool(name="constp", bufs=1) as constp,
            tc.tile_pool(name="sytp", bufs=1) as sytp,
            tc.tile_pool(name="sxtp", bufs=1) as sxtp,
            tc.tile_pool(name="stagep", bufs=3) as stagep,
            tc.tile_pool(name="sgnp", bufs=3) as sgnp,
            tc.tile_pool(name="outp", bufs=4) as outp,
            tc.tile_pool(name="tpsum", bufs=2, space="PSUM") as tpsump,
            tc.tile_pool(name="psump", bufs=6, space="PSUM") as psump,
        ):
            # clip product, replicated per-partition: [P, 1]
            clip_sb = constp.tile([P, 2], f32)
            nc.sync.dma_start(clip_sb[:], clips)
            clip_prod = constp.tile([P, 1], f32)
            nc.vector.tensor_tensor(
                clip_prod[:], clip_sb[:, 0:1], clip_sb[:, 1:2],
                mybir.AluOpType.mult,
            )
            ident = constp.tile([P, P], bf16)
            make_identity(nc, ident[:])

            # SyT[ki, h, kp, n] = sign(y[n, (h*KOH + kp)*P + ki])
            SyT = sytp.tile([P, NHALF, KOH, NSH], op_dt)
            # Persisted x operands: SxT[ki, i, h, kp, m'] for m = i*P + m'
            SxT = sxtp.tile([P, MT, NHALF, KOH, P], op_dt)

            def sign_transpose(src_dram, row, h, dst_slice_fn):
                """DMA a [P, KH] fp32 half-row, apply Sign -> bf16, PE-transpose
                its 16 128x128 blocks (packed KQ per PSUM tile), and DVE-place
                them (cast to op_dt) into dst_slice_fn(q) = [P, KQ, P]."""
                st = stagep.tile([P, KH], f32, name="st", tag="stage")
                nc.sync.dma_start(st[:], src_dram[ts(row, P), ts(h, KH)])
                sg = sgnp.tile([P, KH], bf16, name="sg", tag="sgn")
                nc.scalar.activation(sg[:], st[:], Sign)
                for q in range(KOH // KQ):
                    tp = tpsump.tile([P, KQ * P], bf16, name="tp")
                    for c in range(KQ):
                        nc.tensor.transpose(
                            tp[:, ts(c, P)], sg[:, ts(q * KQ + c, P)], ident[:]
                        )
                    nc.vector.tensor_copy(out=dst_slice_fn(q), in_=tp[:])

            def prep_y_tile(j):
                for h in range(NHALF):
                    sign_transpose(
                        ys, j, h,
                        lambda q, j=j, h=h: SyT[:, h, ts(q, KQ), ts(j, P)],
                    )

            def prep_x_tile(i):
                for h in range(NHALF):
                    sign_transpose(
                        xs, i, h,
                        lambda q, i=i, h=h: SxT[:, i, h, ts(q, KQ), :],
                    )

            def mm(i, nch):
                ps = psump.tile([P, FD], f32, name="ps")
                if USE_FP8:
                    for kd in range(KO // 2):
                        h, kp = divmod(2 * kd, KOH)
                        nc.tensor.matmul(
                            ps[:],
                            lhsT=SxT[:, i, h, kp : kp + 2, :],
                            rhs=SyT[:, h, kp : kp + 2, ts(nch, FD)],
                            start=(kd == 0),
                            stop=(kd == KO // 2 - 1),
                            perf_mode=mybir.MatmulPerfMode.DoubleRow,
                        )
                else:
                    for ko in range(KO):
                        h, kp = divmod(ko, KOH)
                        nc.tensor.matmul(
                            ps[:],
                            lhsT=SxT[:, i, h, kp, :],
                            rhs=SyT[:, h, kp, ts(nch, FD)],
                            start=(ko == 0),
                            stop=(ko == KO - 1),
                        )
                ot = outp.tile([P, FD], f32, name="ot")
                # clip scaling on ScalarE (out = Copy(psum * clip)) — DVE is
                # busy with the cast/placement copies.
                nc.scalar.activation(ot[:], ps[:], Copy, scale=clip_prod[:])
                nc.sync.dma_start(out[ts(i, P), ts(nch, FD)], ot[:])

            # Prologue: first y quarter only.
            for j in range(JPC):
                prep_y_tile(j)

            # Pass 0: x prep (persisted) + matmuls for nch=0, with the
            # remaining y quarters' prep interleaved so it overlaps on
            # DMA/ACT/DVE while the PE chews matmuls.
            for i in range(MT):
                prep_x_tile(i)
                if i % 2 == 0 and JPC + i // 2 < NT:
                    prep_y_tile(JPC + i // 2)
                mm(i, 0)

            for j in range(JPC + MT // 2, NT):
                prep_y_tile(j)

            # Passes 1..3: pure matmul over persisted SxT.
            for nch in range(1, NCH):
                for i in range(MT):
                    mm(i, nch)

    nc.compile()
    return nc


_PROGRAM_CACHE = None


def _get_program():
    global _PROGRAM_CACHE
    if _PROGRAM_CACHE is None:
        _PROGRAM_CACHE = _build_program()
    return _PROGRAM_CACHE


def _shard_inputs(x, y, x_clip, y_clip):
    x = np.asarray(x, dtype=np.float32)
    y = np.asarray(y, dtype=np.float32)
    clips = np.empty((P, 2), dtype=np.float32)
    clips[:, 0] = np.float32(x_clip)
    clips[:, 1] = np.float32(y_clip)
    in_maps = []
    for c in range(NCORES):
        b, mh, nh = c // 4, (c % 4) // 2, c % 2
        in_maps.append(
            {
                "xs": np.ascontiguousarray(x[b, mh * MSH : (mh + 1) * MSH, :]),
                "ys": np.ascontiguousarray(y[b, nh * NSH : (nh + 1) * NSH, :]),
                "clips": clips,
            }
        )
    return in_maps


def run_sharded(x, y, x_clip, y_clip, trace=False, **kwargs):
    """Run the SPMD kernel; returns (out, BassKernelResults)."""
    from concourse.bass_utils import run_bass_kernel_spmd

    nc = _get_program()
    in_maps = _shard_inputs(x, y, x_clip, y_clip)
    res = run_bass_kernel_spmd(
        nc, in_maps, core_ids=list(range(NCORES)), trace=trace, **kwargs
    )
    out = np.empty((B, M, N), dtype=np.float32)
    for c in range(NCORES):
        b, mh, nh = c // 4, (c % 4) // 2, c % 2
        out[b, mh * MSH : (mh + 1) * MSH, nh * NSH : (nh + 1) * NSH] = res.results[
            c
        ]["out"]
    return out, res


def kernel(x, y, x_clip, y_clip):
    out, _ = run_sharded(x, y, x_clip, y_clip, trace=False)
    return out
